# revision 8
# baseline (speedup 1.0000x reference)
"""GATv2 node-feature kernel for Trainium2, SPMD over 8 NeuronCores.

Reference computation (per batch b):
    h = x @ W_w.T + W_b                                  (V, H)
    s_i = h @ a_i ; s_j = h @ a_j                        (V,)   a split per head
    scores[i,j] = leaky_relu(s_i[i] + s_j[j], 0.2) * mean_c(edge_gate[i,j,c])
    weights = softmax_j(scores)
    out = (weights @ h) @ Wo_w.T + Wo_b                  (V, H)

Sharding: 8 cores = 4 batches x 2 halves of the destination-node axis i.
edge_gate (512 MB total) is the dominant tensor; each core streams its own
64 MB shard once (memory-bound roofline ~ shard_bytes / HBM-per-NC-bw).

The SPMD program is identical on every core: per-core inputs are rotated
along the node axis so that each core's destination rows are local rows
0..255 (x and edge_gate's source axis rotated by the same permutation;
softmax and the j-contraction are permutation-invariant).
"""

import sys

if "/opt/trn_rl_repo" not in sys.path:
    sys.path.insert(0, "/opt/trn_rl_repo")

import json
from contextlib import ExitStack

import numpy as np

import concourse.bass as bass
import concourse.mybir as mybir
from concourse.bass_utils import run_bass_kernel_spmd
from concourse.masks import make_identity
from concourse.tile import TileContext

F32 = mybir.dt.float32
AF = mybir.ActivationFunctionType
AX = mybir.AxisListType

B, V, H = 4, 512, 128
NH, HD = 8, 16
IL = V // 2  # destination rows per core
NCORES = 8
NEG_SLOPE = 0.2
JB = 128  # source-node tile width for the edge_gate stream


def _split_multiwait_bir(bir: bytes) -> bytes:
    """Rewrite BIR JSON so no instruction carries more than one sync wait.

    The walrus build in this environment rejects instructions with >1 sync
    waits ("Too many sync wait commands"). Extra waits are hoisted onto
    single-wait NOPs inserted immediately before the instruction on the
    same engine — semantically identical (the engine blocks on each wait
    in sequence before executing the instruction).
    """
    m = json.loads(bir)
    n = 0
    for fn in m["functions"]:
        for blk in fn["blocks"]:
            out = []
            for inst in blk["instructions"]:
                si = inst.get("sync_info")
                waits = (si or {}).get("on_wait") or []
                if len(waits) > 1:
                    for w in waits[:-1]:
                        n += 1
                        out.append(
                            {
                                "debug": inst.get("debug", 0),
                                "engine": inst["engine"],
                                "ins": [],
                                "outs": [],
                                "name": f"waitsplit-{n}",
                                "opcode": "NoOp",
                                "sync_info": {"on_update": [], "on_wait": [w]},
                            }
                        )
                    si["on_wait"] = [waits[-1]]
                out.append(inst)
            blk["instructions"] = out
    return json.dumps(m).encode()


def _patch_serialization(nc: bass.Bass) -> None:
    orig = nc.to_json_bytes
    nc.to_json_bytes = lambda: _split_multiwait_bir(orig())


_CACHE = {}


def build_program() -> bass.Bass:
    if "nc" in _CACHE:
        return _CACHE["nc"]

    nc = bass.Bass()
    eg = nc.declare_dram_parameter("eg", [IL, V, H], F32, isOutput=False)
    xT = nc.declare_dram_parameter("xT", [H, V], F32, isOutput=False)
    wwT = nc.declare_dram_parameter("wwT", [H, H], F32, isOutput=False)
    wb = nc.declare_dram_parameter("wb", [H, 1], F32, isOutput=False)
    apair = nc.declare_dram_parameter("apair", [H, 2], F32, isOutput=False)
    woT = nc.declare_dram_parameter("woT", [H, H], F32, isOutput=False)
    wob = nc.declare_dram_parameter("wob", [H, H], F32, isOutput=False)
    out = nc.declare_dram_parameter("out", [IL, H], F32, isOutput=True)

    njb = V // JB

    with TileContext(nc) as tc, ExitStack() as ctx:
        singles = ctx.enter_context(tc.tile_pool(name="singles", bufs=1))

        xT_sb = singles.tile([H, V], F32)
        nc.sync.dma_start(out=xT_sb, in_=xT[:, :])
        wwT_sb = singles.tile([H, H], F32)
        nc.sync.dma_start(out=wwT_sb, in_=wwT[:, :])
        wb_sb = singles.tile([H, 1], F32)
        nc.sync.dma_start(out=wb_sb, in_=wb[:, :])
        a_sb = singles.tile([H, 2], F32)
        nc.sync.dma_start(out=a_sb, in_=apair[:, :])
        woT_sb = singles.tile([H, H], F32)
        nc.sync.dma_start(out=woT_sb, in_=woT[:, :])
        wob_sb = singles.tile([H, H], F32)
        nc.sync.dma_start(out=wob_sb, in_=wob[:, :])
        ident = singles.tile([128, 128], F32)
        make_identity(nc, ident)

        h_sb = singles.tile([H, V], F32)  # h^T: (feature, node)
        si_row = singles.tile([1, V], F32)
        sj_row = singles.tile([1, V], F32)
        ones_row = singles.tile([1, V], F32)
        h_col = [singles.tile([128, H], F32, tag=f"hcol{j}", name=f"hcol{j}") for j in range(4)]
        pre = [singles.tile([128, V], F32, tag=f"pre{t}", name=f"pre{t}") for t in range(2)]

        nc.vector.memset(ones_row, 1.0)

        with tc.tile_pool(name="psetup", bufs=1, space="PSUM") as pst:
            # h^T = W_w @ x^T  (+ bias per feature-partition)
            ph = pst.tile([H, V], F32)
            nc.tensor.matmul(ph, lhsT=wwT_sb, rhs=xT_sb, start=True, stop=True)
            nc.scalar.activation(h_sb, ph, AF.Identity, bias=wb_sb)

            # s_i/s_j rows: (1, V) = a_{i,j}^T @ h^T
            psi = pst.tile([1, V], F32)
            nc.tensor.matmul(psi, lhsT=a_sb[:, 0:1], rhs=h_sb, start=True, stop=True)
            nc.scalar.copy(si_row, psi)
            psj = pst.tile([1, V], F32)
            nc.tensor.matmul(psj, lhsT=a_sb[:, 1:2], rhs=h_sb, start=True, stop=True)
            nc.scalar.copy(sj_row, psj)

            # h in (node, feature) layout for the attention contraction
            for jc in range(4):
                th = pst.tile([128, 128], F32, tag="th")
                nc.tensor.transpose(th, h_sb[:, jc * 128 : (jc + 1) * 128], ident)
                nc.scalar.copy(h_col[jc], th)

            # pre[t][i, j] = leaky_relu(s_i[i] + s_j[j]) via two rank-1
            # accumulating matmuls (outer sum); leaky = max(z, 0.2z) on DVE
            for t in range(2):
                pp = pst.tile([128, V], F32, tag="pp")
                nc.tensor.matmul(
                    pp,
                    lhsT=si_row[0:1, t * 128 : (t + 1) * 128],
                    rhs=ones_row,
                    start=True,
                    stop=False,
                )
                nc.tensor.matmul(
                    pp,
                    lhsT=ones_row[0:1, 0:128],
                    rhs=sj_row,
                    start=False,
                    stop=True,
                )
                zc = singles.tile([128, V], F32, tag=f"zc{t}", name=f"zc{t}")
                nc.scalar.copy(zc, pp)
                nc.vector.scalar_tensor_tensor(
                    out=pre[t],
                    in0=zc,
                    scalar=NEG_SLOPE,
                    in1=zc,
                    op0=mybir.AluOpType.mult,
                    op1=mybir.AluOpType.max,
                )

        egp = ctx.enter_context(tc.tile_pool(name="egp", bufs=2))
        work = ctx.enter_context(tc.tile_pool(name="work", bufs=2))
        pmain = ctx.enter_context(tc.tile_pool(name="pmain", bufs=2, space="PSUM"))
        outp = ctx.enter_context(tc.tile_pool(name="outp", bufs=2))

        for t in range(2):
            r0 = t * 128
            mask = work.tile([128, V], F32, tag="mask")
            for jb in range(njb):
                egt = egp.tile([128, JB, H], F32, tag="egt")
                nc.sync.dma_start(
                    out=egt, in_=eg[r0 : r0 + 128, jb * JB : (jb + 1) * JB, :]
                )
                nc.vector.reduce_sum(
                    mask[:, jb * JB : (jb + 1) * JB], egt, axis=AX.X
                )

            # scores*128; the 1/H mean factor is folded into the exp scale
            tt = work.tile([128, V], F32, tag="tt")
            nc.vector.tensor_mul(tt, pre[t], mask)
            mx = work.tile([128, 1], F32, tag="mx")
            nc.vector.reduce_max(mx, tt, axis=AX.X)
            nmx = work.tile([128, 1], F32, tag="nmx")
            nc.scalar.mul(nmx, mx, -1.0 / H)
            ew = work.tile([128, V], F32, tag="ew")
            z = work.tile([128, 1], F32, tag="z")
            nc.scalar.activation(
                ew, tt, AF.Exp, bias=nmx, scale=1.0 / H, accum_out=z
            )
            rcp = work.tile([128, 1], F32, tag="rcp")
            nc.vector.reciprocal(rcp, z)

            # attended^T = sum_jc h_col[jc]^T-contraction with softmax weights
            pa = pmain.tile([128, 128], F32, tag="pa")
            for jc in range(4):
                tw = pmain.tile([128, 128], F32, tag="tw")
                nc.tensor.transpose(tw, ew[:, jc * 128 : (jc + 1) * 128], ident)
                tw_sb = work.tile([128, 128], F32, tag="tw_sb")
                nc.scalar.copy(tw_sb, tw)
                nc.tensor.matmul(
                    pa, lhsT=h_col[jc], rhs=tw_sb, start=(jc == 0), stop=(jc == 3)
                )
            attT = work.tile([128, 128], F32, tag="attT")
            nc.scalar.copy(attT, pa)

            # out = (attended/Z) @ Wo_w.T + Wo_b
            po = pmain.tile([128, 128], F32, tag="po")
            nc.tensor.matmul(po, lhsT=attT, rhs=woT_sb, start=True, stop=True)
            sc = outp.tile([128, H], F32, tag="sc")
            nc.scalar.activation(sc, po, AF.Copy, scale=rcp)
            ob = outp.tile([128, H], F32, tag="ob")
            nc.vector.tensor_add(ob, sc, wob_sb)
            nc.sync.dma_start(out=out[r0 : r0 + 128, :], in_=ob)

    _patch_serialization(nc)
    _CACHE["nc"] = nc
    return nc


def make_in_maps(inputs: dict) -> list[dict]:
    x = np.ascontiguousarray(np.asarray(inputs["x"], dtype=np.float32))
    eg = np.asarray(inputs["edge_gate"], dtype=np.float32)
    W_w = np.asarray(inputs["W_w"], dtype=np.float32)
    W_b = np.asarray(inputs["W_b"], dtype=np.float32)
    a = np.asarray(inputs["a"], dtype=np.float32)
    Wo_w = np.asarray(inputs["Wo_w"], dtype=np.float32)
    Wo_b = np.asarray(inputs["Wo_b"], dtype=np.float32)

    a_i = np.ascontiguousarray(a[0, :, :HD]).reshape(H)
    a_j = np.ascontiguousarray(a[0, :, HD:]).reshape(H)
    apair = np.ascontiguousarray(np.stack([a_i, a_j], axis=1))
    wwT = np.ascontiguousarray(W_w.T)
    woT = np.ascontiguousarray(Wo_w.T)
    wb = np.ascontiguousarray(W_b.reshape(H, 1))
    wob = np.ascontiguousarray(np.broadcast_to(Wo_b[None, :], (H, H)))

    in_maps = []
    for c in range(NCORES):
        b, ih = divmod(c, 2)
        i0 = ih * IL
        xTl = np.ascontiguousarray(np.roll(x[b].T, -i0, axis=1))
        egl = eg[b, i0 : i0 + IL]
        if i0:
            egl = np.roll(egl, -i0, axis=1)
        in_maps.append(
            {
                "eg": np.ascontiguousarray(egl),
                "xT": xTl,
                "wwT": wwT,
                "wb": wb,
                "apair": apair,
                "woT": woT,
                "wob": wob,
            }
        )
    return in_maps


def gather_out(results: list[dict]) -> np.ndarray:
    out = np.empty((B, V, H), dtype=np.float32)
    for c in range(NCORES):
        b, ih = divmod(c, 2)
        out[b, ih * IL : (ih + 1) * IL] = results[c]["out"]
    return out


def kernel(**inputs) -> np.ndarray:
    nc = build_program()
    in_maps = make_in_maps(inputs)
    res = run_bass_kernel_spmd(nc, in_maps, list(range(NCORES)))
    return gather_out(res.results)
